# revision 1
# baseline (speedup 1.0000x reference)
"""Trainium2 Bass kernel for nn_CategoricalLinear (MoE-routing batched matvec).

Problem: out[b] = weight[selected_ids[b]] @ x[b]
  x: [2048, 512] f32, selected_ids: [2048] int, weight: [64, 512, 512] f32
  out: [2048, 512] f32

Strategy (category-sharded, NOT the data-parallel hint):
  - Host: stable-sort samples by category; category c's samples become a
    contiguous block. Transpose x so features lie on SBUF partitions.
  - Each of the 8 cores owns 8 categories (8 MB weight slab — the minimal
    1/8 slice of the 64 MB table) and ALL samples routed to them (~256).
  - Per category g: out_g[s, o] = sum_i x[s, i] * W_g[o, i] computed as
    4 accumulating PE matmuls: stationary = xT chunk [128(K=IN), PC(samples)],
    moving = W_g^T chunk [128(K=IN), 512(OUT)], PSUM [PC, 512].
    float32r data path -> full-rate PE (fp32 would stream at 1/4 rate).
  - Weight slab streamed per-category (1 MB DMAs) and double-buffered so the
    PE and the output path hide entirely under the weight DMA (~8 MB/core,
    the bandwidth floor for this sharding).
  - Host: unpad + inverse-permute rows back to the original sample order.

This is better than data-parallel replication: sharding the batch would make
every core read ~the whole 64 MB table (8x the aggregate HBM traffic) and
leaves ~4 samples per (core, category) matmul.
"""

import numpy as np

B, IN, OUT, C = 2048, 512, 512, 64
NCORES = 8
CPC = C // NCORES  # categories per core
KCH = IN // 128  # contraction chunks of 128


def _build_nc(
    PC,
    mm_dtype: str = "float32r",
    loop_iters: int = 0,
    unroll: int = 1,
    wbufs: int = 4,
    cats_per_dma: int = 1,
    interleave: bool = False,
    alt_rings: bool = False,
    split_first: bool = False,
    w_engine: str = "sync",
    merge_xt: bool = False,
    ppbufs: int = 4,
    opbufs: int = 3,
):
    """Build + compile the SPMD Bass program (same NEFF runs on all 8 cores).

    PC: per-slot sample capacities (even, <= 128) — an int (uniform) or a
        sequence of CPC values. Slot g on every core holds one category
        padded to PC[g] samples.
    loop_iters: if > 0, wrap the body in a device-side For_i loop with
        `unroll` copies of the body per iteration (timing use only).
    """
    import concourse.mybir as mybir
    import concourse.tile as tile
    from concourse import bacc

    f32 = mybir.dt.float32
    mmdt = getattr(mybir.dt, mm_dtype)
    PCs = [PC] * CPC if isinstance(PC, int) else list(PC)
    assert len(PCs) == CPC
    SOFF = [0]
    for p in PCs:
        SOFF.append(SOFF[-1] + p)
    NCOL = SOFF[-1]

    nc = bacc.Bacc(
        "TRN2", target_bir_lowering=False, debug=False, num_devices=NCORES
    )
    wt = nc.dram_tensor("wt", [CPC * IN, OUT], mmdt, kind="ExternalInput").ap()
    xt = nc.dram_tensor("xt", [IN, NCOL], mmdt, kind="ExternalInput").ap()
    out = nc.dram_tensor("out", [NCOL, OUT], f32, kind="ExternalOutput").ap()

    with tile.TileContext(nc) as tc:
        with (
            tc.tile_pool(name="xp", bufs=1) as xp,
            tc.tile_pool(name="wp", bufs=wbufs) as wp,
            tc.tile_pool(name="pp", bufs=ppbufs, space="PSUM") as pp,
            tc.tile_pool(name="op", bufs=opbufs) as op,
        ):

            def body():
                G = cats_per_dma
                if interleave:
                    # p-outer row mapping: partition p holds IN rows
                    # KCH*p + s (s=0..KCH-1). Every DMA is contiguous per
                    # partition (8 KB weight runs, one single xT DMA); the
                    # contraction over s-subsets is a row permutation the
                    # matmul accumulation doesn't care about, as long as x
                    # and W use the same mapping.
                    xt4 = xp.tile([128, KCH, NCOL], mmdt, tag="x4")
                    nc.scalar.dma_start(
                        out=xt4[:], in_=xt.rearrange("(p s) c -> p s c", p=128)
                    )
                    lhs = lambda s, g: xt4[:, s, SOFF[g] : SOFF[g] + PCs[g]]
                elif merge_xt:
                    # One 3-D DMA for all four k-chunks (same k-outer layout,
                    # one descriptor chain / one fixed cost on the fill path).
                    xt1 = xp.tile([128, KCH, NCOL], mmdt, tag="x1")
                    nc.scalar.dma_start(
                        out=xt1[:], in_=xt.rearrange("(k p) c -> p k c", p=128)
                    )
                    lhs = lambda s, g: xt1[:, s, SOFF[g] : SOFF[g] + PCs[g]]
                else:
                    xts = []
                    for k in range(KCH):
                        t = xp.tile([128, NCOL], mmdt, tag=f"x{k}")
                        # ACT ring: keep SP HWDGE free for the weight stream
                        nc.scalar.dma_start(
                            out=t[:], in_=xt[k * 128 : (k + 1) * 128, :]
                        )
                        xts.append(t)
                    lhs = lambda s, g: xts[s][:, SOFF[g] : SOFF[g] + PCs[g]]
                for gp in range(0, CPC, G):
                    # Weight block [G cats] as SBUF [128, G, KCH, OUT]. G MB/DMA.
                    wtile = wp.tile([128, G, KCH, OUT], mmdt)
                    if interleave:
                        src = wt[gp * IN : (gp + G) * IN, :].rearrange(
                            "(g p s) o -> p g s o", p=128, s=KCH
                        )
                    else:
                        src = wt[gp * IN : (gp + G) * IN, :].rearrange(
                            "(g k p) o -> p g k o", p=128, k=KCH
                        )
                    weng = (
                        nc.scalar
                        if (alt_rings and (gp // G) % 2)
                        else getattr(nc, w_engine)
                    )
                    if split_first and gp == 0 and G == 1:
                        # Halve the fill latency: the first two matmuls only
                        # need k-chunks 0-1, so land them in their own DMA.
                        half = wp.tile([128, 1, KCH // 2, OUT], mmdt, tag="wh")
                        weng.dma_start(
                            out=half[:],
                            in_=wt[0 : IN // 2, :].rearrange(
                                "(g k p) o -> p g k o", p=128, k=KCH // 2
                            ),
                        )
                        weng.dma_start(
                            out=wtile[:, :, KCH // 2 :, :],
                            in_=wt[IN // 2 : IN, :].rearrange(
                                "(g k p) o -> p g k o", p=128, k=KCH // 2
                            ),
                        )
                        first_half = half
                    else:
                        weng.dma_start(out=wtile[:], in_=src)
                        first_half = None
                    for gl in range(G):
                        g = gp + gl
                        ps = pp.tile([PCs[g], OUT], f32, tag="ps")
                        for k in range(KCH):
                            if first_half is not None and k < KCH // 2:
                                rhs = first_half[:, gl, k, :]
                            else:
                                rhs = wtile[:, gl, k, :]
                            nc.tensor.matmul(
                                ps[:],
                                lhsT=lhs(k, g),
                                rhs=rhs,
                                start=(k == 0),
                                stop=(k == KCH - 1),
                            )
                        ot = op.tile([PCs[g], OUT], f32, tag="ot")
                        nc.vector.tensor_copy(out=ot[:], in_=ps[:])
                        nc.scalar.dma_start(
                            out=out[SOFF[g] : SOFF[g] + PCs[g], :], in_=ot[:]
                        )

            if loop_iters > 0:
                with tc.For_i(0, loop_iters, 1):
                    for _ in range(unroll):
                        body()
            else:
                for _ in range(unroll):
                    body()
    nc.compile()
    return nc


def _prepare(x, selected_ids, weight, mm_dtype="float32r"):
    """Host-side shard prep. Returns (in_maps, meta), or (None, None) when the
    inputs don't fit the compiled layout (handled by the host fallback)."""
    host_dt = np.float16 if mm_dtype == "float16" else np.float32
    x = np.ascontiguousarray(np.asarray(x, dtype=np.float32))
    ids = np.asarray(selected_ids).astype(np.int64).ravel()
    weight = np.asarray(weight, dtype=np.float32)
    if ids.size != B or ids.min() < 0 or ids.max() >= C:
        return None, None  # out-of-range ids -> host path
    counts = np.bincount(ids, minlength=C)
    mx = int(counts.max())
    if mx > 128 or weight.shape != (C, OUT, IN) or x.shape != (B, IN):
        return None, None  # pathological skew / unexpected shape -> host path
    order = np.argsort(ids, kind="stable")
    x_sorted = x[order]
    offs = np.zeros(C + 1, np.int64)
    offs[1:] = np.cumsum(counts)
    # Identity assignment, uniform capacity rounded to 16. Measured fastest on
    # HW: sorted-assignment layouts with tighter per-slot capacities moved
    # ~0.5 MB/core less but ran 0.9-1.5 us slower (shorter DMA runs / smaller
    # output blocks cost more than the saved bytes). Capacity must be EVEN or
    # the fp32r matmul fast path degrades ~2x (PC=43 measured 58.8 us).
    assign = np.arange(C).reshape(NCORES, CPC).T  # [slot, core] -> category
    PCs = [min(128, max(16, (mx + 15) // 16 * 16))] * CPC
    SOFF = np.zeros(CPC + 1, np.int64)
    SOFF[1:] = np.cumsum(PCs)
    NCOL = int(SOFF[-1])
    wt_t = np.ascontiguousarray(weight.transpose(0, 2, 1).astype(host_dt))
    in_maps = []
    for core in range(NCORES):
        xt_k = np.zeros((IN, NCOL), host_dt)
        wlist = []
        for g in range(CPC):
            c = int(assign[g, core])
            n = int(counts[c])
            if n:
                xt_k[:, SOFF[g] : SOFF[g] + n] = (
                    x_sorted[offs[c] : offs[c + 1]].T.astype(host_dt)
                )
            wlist.append(wt_t[c])
        w_k = np.concatenate(wlist, axis=0)  # [CPC*IN, OUT]
        in_maps.append({"wt": w_k, "xt": xt_k})
    meta = dict(
        PCs=PCs, SOFF=SOFF, assign=assign, counts=counts, offs=offs, order=order
    )
    return in_maps, meta


def _gather(results, meta):
    counts, offs, order = meta["counts"], meta["offs"], meta["order"]
    assign, SOFF = meta["assign"], meta["SOFF"]
    out_sorted = np.empty((B, OUT), np.float32)
    for core in range(NCORES):
        o = results[core]["out"]
        for g in range(CPC):
            c = int(assign[g, core])
            n = int(counts[c])
            if n:
                out_sorted[offs[c] : offs[c + 1]] = o[SOFF[g] : SOFF[g] + n]
    out_full = np.empty_like(out_sorted)
    out_full[order] = out_sorted
    return out_full


_LAST = {}  # debug/test introspection: last built nc + shard maps


def kernel(x, selected_ids, weight):
    in_maps, meta = _prepare(x, selected_ids, weight)
    if in_maps is None:
        # Host fallback for inputs outside the compiled layout's assumptions.
        ids = np.asarray(selected_ids).astype(np.int64).ravel()
        w = np.asarray(weight, dtype=np.float32)
        xx = np.asarray(x, dtype=np.float32).reshape(ids.size, -1)
        outf = np.empty((ids.size, w.shape[1]), np.float32)
        for c in np.unique(ids):
            m = ids == c
            outf[m] = xx[m] @ w[c].T
        return outf
    from concourse.bass_utils import run_bass_kernel_spmd

    # float32: exact f32-class result (fro 1.2e-07 vs f64), measured 34.66 us
    # with wbufs=6 (vs float32r's 32.22 us / 1.25e-04 and float16's
    # 21.89 us / 2.50e-04). Exactness buys zero numerical-threshold risk.
    # wbufs=6: fp32 is PE-paced (4 cyc/row), so deeper weight lookahead wins
    # (-0.7 us); in the DMA-paced fp32r/fp16 regimes it measured worse.
    nc = _build_nc(meta["PCs"], mm_dtype="float32", wbufs=6)
    _LAST.update(nc=nc, in_maps=in_maps, meta=meta)
    res = run_bass_kernel_spmd(nc, in_maps, core_ids=list(range(NCORES)))
    return _gather(res.results, meta)



# revision 29
# speedup vs baseline: 1.4373x; 1.4373x over previous
"""Trainium2 Bass kernel for nn_CategoricalLinear (MoE-routing batched matvec).

Problem: out[b] = weight[selected_ids[b]] @ x[b]
  x: [2048, 512] f32, selected_ids: [2048] int, weight: [64, 512, 512] f32
  out: [2048, 512] f32

Strategy (category-sharded, NOT the data-parallel hint):
  - Host: stable-sort samples by category; category c's samples become a
    contiguous block. Transpose x so features lie on SBUF partitions.
  - Each of the 8 cores owns 8 categories (8 MB weight slab — the minimal
    1/8 slice of the 64 MB table) and ALL samples routed to them (~256).
  - Per category g: out_g[s, o] = sum_i x[s, i] * W_g[o, i] computed as
    4 accumulating PE matmuls: stationary = xT chunk [128(K=IN), PC(samples)],
    moving = W_g^T chunk [128(K=IN), 512(OUT)], PSUM [PC, 512].
    float32r data path -> full-rate PE (fp32 would stream at 1/4 rate).
  - Weight slab streamed per-category (1 MB DMAs) and double-buffered so the
    PE and the output path hide entirely under the weight DMA (~8 MB/core,
    the bandwidth floor for this sharding).
  - Host: unpad + inverse-permute rows back to the original sample order.

This is better than data-parallel replication: sharding the batch would make
every core read ~the whole 64 MB table (8x the aggregate HBM traffic) and
leaves ~4 samples per (core, category) matmul.
"""

import numpy as np

B, IN, OUT, C = 2048, 512, 512, 64
NCORES = 8
CPC = C // NCORES  # categories per core
KCH = IN // 128  # contraction chunks of 128


def _build_nc(
    PC,
    mm_dtype: str = "float32r",
    loop_iters: int = 0,
    unroll: int = 1,
    wbufs: int = 4,
    cats_per_dma: int = 1,
    interleave: bool = False,
    alt_rings: bool = False,
    split_first: bool = False,
    w_engine: str = "sync",
    merge_xt: bool = False,
    ppbufs: int = 4,
    opbufs: int = 3,
    out_dtype: str = "float32",
    wsplit: int = 1,
    x_engine: str = "scalar",
    o_engine: str = "scalar",
    copy_engine: str = "vector",
    xbufs: int = 1,
    diag: str = "",  # "wonly": weight DMAs only; "noout": skip copy+out
    obatch: int = 1,  # cats per output tile/DMA (obatch*PC <= 128 rows)
    linear: bool = False,  # host pre-linearized DRAM layouts (pure memcpy DMAs)
):
    """Build + compile the SPMD Bass program (same NEFF runs on all 8 cores).

    PC: per-slot sample capacities (even, <= 128) — an int (uniform) or a
        sequence of CPC values. Slot g on every core holds one category
        padded to PC[g] samples.
    loop_iters: if > 0, wrap the body in a device-side For_i loop with
        `unroll` copies of the body per iteration (timing use only).
    """
    import concourse.mybir as mybir
    import concourse.tile as tile
    from concourse import bacc

    f32 = mybir.dt.float32
    mmdt = getattr(mybir.dt, mm_dtype)
    odt = getattr(mybir.dt, out_dtype)
    PCs = [PC] * CPC if isinstance(PC, int) else list(PC)
    assert len(PCs) == CPC
    assert wsplit == 1 or cats_per_dma == 1
    SOFF = [0]
    for p in PCs:
        SOFF.append(SOFF[-1] + p)
    NCOL = SOFF[-1]

    nc = bacc.Bacc(
        "TRN2", target_bir_lowering=False, debug=False, num_devices=NCORES
    )
    if linear:
        # DRAM mirrors the SBUF destination layout (p-outer, feature
        # i = 4p+s): every DMA degenerates to a per-partition contiguous
        # copy (8-32 KB runs) with zero strided descriptors.
        assert all(p == PCs[0] for p in PCs)
        PCU = PCs[0]
        wt = nc.dram_tensor(
            "wt", [128, CPC, KCH, OUT], mmdt, kind="ExternalInput"
        ).ap()
        xt = nc.dram_tensor(
            "xt", [128, KCH, NCOL], mmdt, kind="ExternalInput"
        ).ap()
        out = nc.dram_tensor(
            "out", [PCU, CPC, OUT], odt, kind="ExternalOutput"
        ).ap()
    else:
        wt = nc.dram_tensor(
            "wt", [CPC * IN, OUT], mmdt, kind="ExternalInput"
        ).ap()
        xt = nc.dram_tensor("xt", [IN, NCOL], mmdt, kind="ExternalInput").ap()
        out = nc.dram_tensor("out", [NCOL, OUT], odt, kind="ExternalOutput").ap()

    with tile.TileContext(nc) as tc:
        with (
            tc.tile_pool(name="xp", bufs=xbufs) as xp,
            tc.tile_pool(name="wp", bufs=wbufs) as wp,
            tc.tile_pool(name="pp", bufs=ppbufs, space="PSUM") as pp,
            tc.tile_pool(name="op", bufs=opbufs) as op,
        ):

            def body():
                G = cats_per_dma
                xdma = getattr(nc, x_engine).dma_start
                if diag == "wonly":
                    lhs = None
                elif linear:
                    xtl = xp.tile([128, KCH, NCOL], mmdt, tag="xl")
                    if diag != "nox":  # nox: tile without fill (timing diag)
                        xdma(out=xtl[:], in_=xt[:])
                    lhs = lambda s, g: xtl[:, s, SOFF[g] : SOFF[g] + PCs[g]]
                elif interleave:
                    # p-outer row mapping: partition p holds IN rows
                    # KCH*p + s (s=0..KCH-1). Every DMA is contiguous per
                    # partition (8 KB weight runs, one single xT DMA); the
                    # contraction over s-subsets is a row permutation the
                    # matmul accumulation doesn't care about, as long as x
                    # and W use the same mapping.
                    xt4 = xp.tile([128, KCH, NCOL], mmdt, tag="x4")
                    xdma(
                        out=xt4[:], in_=xt.rearrange("(p s) c -> p s c", p=128)
                    )
                    lhs = lambda s, g: xt4[:, s, SOFF[g] : SOFF[g] + PCs[g]]
                elif merge_xt:
                    # One 3-D DMA for all four k-chunks (same k-outer layout,
                    # one descriptor chain / one fixed cost on the fill path).
                    xt1 = xp.tile([128, KCH, NCOL], mmdt, tag="x1")
                    xdma(
                        out=xt1[:], in_=xt.rearrange("(k p) c -> p k c", p=128)
                    )
                    lhs = lambda s, g: xt1[:, s, SOFF[g] : SOFF[g] + PCs[g]]
                else:
                    xts = []
                    for k in range(KCH):
                        t = xp.tile([128, NCOL], mmdt, tag=f"x{k}")
                        # ACT ring: keep SP HWDGE free for the weight stream
                        xdma(
                            out=t[:], in_=xt[k * 128 : (k + 1) * 128, :]
                        )
                        xts.append(t)
                    lhs = lambda s, g: xts[s][:, SOFF[g] : SOFF[g] + PCs[g]]
                xeng = getattr(nc, x_engine)
                oeng = getattr(nc, o_engine)
                ceng = getattr(nc, copy_engine)
                ndma = 0  # weight-DMA issue counter (for alt_rings)
                for gp in range(0, CPC, G):
                    # Weight block [G cats] as SBUF [128, G, KCH, OUT]. G MB/DMA.
                    wtile = wp.tile([128, G, KCH, OUT], mmdt)
                    if linear:
                        src = wt[:, gp : gp + G, :, :]
                    elif interleave:
                        src = wt[gp * IN : (gp + G) * IN, :].rearrange(
                            "(g p s) o -> p g s o", p=128, s=KCH
                        )
                    else:
                        src = wt[gp * IN : (gp + G) * IN, :].rearrange(
                            "(g k p) o -> p g k o", p=128, k=KCH
                        )

                    def weng():
                        nonlocal ndma
                        ndma += 1
                        return (
                            nc.scalar
                            if (alt_rings and (ndma - 1) % 2)
                            else getattr(nc, w_engine)
                        )

                    first_half = None
                    if wsplit > 1:
                        # Split the per-category weight DMA into wsplit
                        # k-groups (finer DMA/matmul interleaving).
                        kg = KCH // wsplit
                        for h in range(wsplit):
                            if interleave:
                                sub = wt[gp * IN : (gp + 1) * IN, :].rearrange(
                                    "(p s) o -> p s o", p=128
                                )[:, h * kg : (h + 1) * kg, :]
                            else:
                                sub = wt[
                                    gp * IN + h * kg * 128 : gp * IN
                                    + (h + 1) * kg * 128,
                                    :,
                                ].rearrange("(k p) o -> p k o", p=128)
                            weng().dma_start(
                                out=wtile[:, 0, h * kg : (h + 1) * kg, :],
                                in_=sub,
                            )
                    elif split_first and gp == 0 and G == 1:
                        # Halve the fill latency: the first two matmuls only
                        # need k-chunks 0-1, so land them in their own DMA.
                        half = wp.tile([128, 1, KCH // 2, OUT], mmdt, tag="wh")
                        weng().dma_start(
                            out=half[:],
                            in_=wt[0 : IN // 2, :].rearrange(
                                "(g k p) o -> p g k o", p=128, k=KCH // 2
                            ),
                        )
                        weng().dma_start(
                            out=wtile[:, :, KCH // 2 :, :],
                            in_=wt[IN // 2 : IN, :].rearrange(
                                "(g k p) o -> p g k o", p=128, k=KCH // 2
                            ),
                        )
                        first_half = half
                    else:
                        weng().dma_start(out=wtile[:], in_=src)
                    if diag == "wonly":
                        continue
                    for gl in range(G):
                        g = gp + gl
                        ps = pp.tile([PCs[g], OUT], f32, tag="ps")
                        for k in range(KCH):
                            if first_half is not None and k < KCH // 2:
                                rhs = first_half[:, gl, k, :]
                            else:
                                rhs = wtile[:, gl, k, :]
                            nc.tensor.matmul(
                                ps[:],
                                lhsT=lhs(k, g),
                                rhs=rhs,
                                start=(k == 0),
                                stop=(k == KCH - 1),
                            )
                        if diag == "noout":
                            continue
                        # Batch obatch consecutive cats into one SBUF tile
                        # (packed along the free dim — partition offsets on
                        # DVE writes must be 32-aligned, free offsets are
                        # unconstrained) and one 3-dim store DMA.
                        g0 = (g // obatch) * obatch
                        ob = obatch
                        if ob > 1:
                            assert all(p == PCs[0] for p in PCs)
                            if g % ob == 0:
                                body.ot = op.tile(
                                    [PCs[g], ob, OUT], odt, tag="ot"
                                )
                            ceng.tensor_copy(
                                out=body.ot[:, g - g0, :], in_=ps[:]
                            )
                            if g % ob == ob - 1:
                                if linear:
                                    dst = out[:, g0 : g0 + ob, :]
                                else:
                                    dst = out[
                                        SOFF[g0] : SOFF[g0] + ob * PCs[g], :
                                    ].rearrange("(b p) o -> p b o", b=ob)
                                oeng.dma_start(out=dst, in_=body.ot[:])
                        else:
                            ot = op.tile([PCs[g], OUT], odt, tag="ot")
                            ceng.tensor_copy(out=ot[:], in_=ps[:])
                            dst = (
                                out[:, g, :]
                                if linear
                                else out[SOFF[g] : SOFF[g] + PCs[g], :]
                            )
                            oeng.dma_start(out=dst, in_=ot[:])

            if loop_iters > 0:
                with tc.For_i(0, loop_iters, 1):
                    for _ in range(unroll):
                        body()
            else:
                for _ in range(unroll):
                    body()
    nc.compile()
    return nc


def _prepare(
    x, selected_ids, weight, mm_dtype="float32r", linear=False, pc_round=16
):
    """Host-side shard prep. Returns (in_maps, meta), or (None, None) when the
    inputs don't fit the compiled layout (handled by the host fallback).

    linear=True emits the pre-linearized layouts matching
    _build_nc(linear=True): wt [128, CPC, KCH, OUT] (p-outer, feature
    i = KCH*p + s), xt [128, KCH, NCOL], out [PC, CPC, OUT]."""
    host_dt = np.float16 if mm_dtype == "float16" else np.float32
    x = np.ascontiguousarray(np.asarray(x, dtype=np.float32))
    ids = np.asarray(selected_ids).astype(np.int64).ravel()
    weight = np.asarray(weight, dtype=np.float32)
    if ids.size != B or ids.min() < 0 or ids.max() >= C:
        return None, None  # out-of-range ids -> host path
    counts = np.bincount(ids, minlength=C)
    mx = int(counts.max())
    if mx > 128 or weight.shape != (C, OUT, IN) or x.shape != (B, IN):
        return None, None  # pathological skew / unexpected shape -> host path
    order = np.argsort(ids, kind="stable")
    x_sorted = x[order]
    offs = np.zeros(C + 1, np.int64)
    offs[1:] = np.cumsum(counts)
    # Identity assignment, uniform capacity rounded to 16. Measured fastest on
    # HW: sorted-assignment layouts with tighter per-slot capacities moved
    # ~0.5 MB/core less but ran 0.9-1.5 us slower (shorter DMA runs / smaller
    # output blocks cost more than the saved bytes). Capacity must be EVEN or
    # the fp32r matmul fast path degrades ~2x (PC=43 measured 58.8 us).
    assign = np.arange(C).reshape(NCORES, CPC).T  # [slot, core] -> category
    r = pc_round
    PCs = [min(128, max(16, (mx + r - 1) // r * r))] * CPC
    SOFF = np.zeros(CPC + 1, np.int64)
    SOFF[1:] = np.cumsum(PCs)
    NCOL = int(SOFF[-1])
    wt_t = np.ascontiguousarray(weight.transpose(0, 2, 1).astype(host_dt))
    in_maps = []
    for core in range(NCORES):
        xt_k = np.zeros((IN, NCOL), host_dt)
        wlist = []
        for g in range(CPC):
            c = int(assign[g, core])
            n = int(counts[c])
            if n:
                xt_k[:, SOFF[g] : SOFF[g] + n] = (
                    x_sorted[offs[c] : offs[c + 1]].T.astype(host_dt)
                )
            wlist.append(wt_t[c])
        if linear:
            # [g][i=4p+s, o] -> [p, g, s, o]
            w_k = np.ascontiguousarray(
                np.stack(wlist, 0)
                .reshape(CPC, 128, KCH, OUT)
                .transpose(1, 0, 2, 3)
            )
            xt_k = np.ascontiguousarray(xt_k.reshape(128, KCH, NCOL))
        else:
            w_k = np.concatenate(wlist, axis=0)  # [CPC*IN, OUT]
        in_maps.append({"wt": w_k, "xt": xt_k})
    meta = dict(
        PCs=PCs, SOFF=SOFF, assign=assign, counts=counts, offs=offs,
        order=order, linear=linear,
    )
    return in_maps, meta


def _gather(results, meta):
    counts, offs, order = meta["counts"], meta["offs"], meta["order"]
    assign, SOFF = meta["assign"], meta["SOFF"]
    out_sorted = np.empty((B, OUT), np.float32)
    for core in range(NCORES):
        o = results[core]["out"]
        for g in range(CPC):
            c = int(assign[g, core])
            n = int(counts[c])
            if n:
                blk = o[:n, g] if meta.get("linear") else o[SOFF[g] : SOFF[g] + n]
                out_sorted[offs[c] : offs[c + 1]] = blk
    out_full = np.empty_like(out_sorted)
    out_full[order] = out_sorted
    return out_full


_LAST = {}  # debug/test introspection: last built nc + shard maps

# Measured-best build config (loop-slope HW timing, 2026-08-08):
#   float16 weights/x (fro 2.5e-4 vs f64 — 80x inside the 2e-2 gate) halve
#   the dominant weight stream vs f32; fp16 output halves the store.
#   linear: host pre-linearizes DRAM to the SBUF destination layout, so
#   every DMA is a per-partition contiguous copy (weight stream measured
#   335 GB/s — at the ~332 GB/s effective HBM ceiling).
#   xbufs=2: double-buffered x tile; body i+1's x DMA overlaps body i's
#   tail instead of stalling the weight stream (-4.5 us, the single
#   biggest win).  cats_per_dma=2: 1 MB weight DMAs.  obatch=8: all 8
#   output blocks packed along the free dim of one SBUF tile -> a single
#   store DMA per body.  Measured 15.9-16.4 us/body (vs 32.2 baseline);
#   body decomposition: weights 12.5 + x 1.1 + out 1.1 + loop-sync ~1.3.
BEST_CFG = dict(
    mm_dtype="float16",
    out_dtype="float16",
    linear=True,
    xbufs=2,
    cats_per_dma=2,
    obatch=8,
    wbufs=6,
    opbufs=2,
)


def kernel(x, selected_ids, weight):
    in_maps, meta = _prepare(
        x,
        selected_ids,
        weight,
        mm_dtype=BEST_CFG["mm_dtype"],
        linear=BEST_CFG.get("linear", False),
    )
    if in_maps is None:
        # Host fallback for inputs outside the compiled layout's assumptions.
        ids = np.asarray(selected_ids).astype(np.int64).ravel()
        w = np.asarray(weight, dtype=np.float32)
        xx = np.asarray(x, dtype=np.float32).reshape(ids.size, -1)
        outf = np.empty((ids.size, w.shape[1]), np.float32)
        for c in np.unique(ids):
            m = ids == c
            outf[m] = xx[m] @ w[c].T
        return outf
    from concourse.bass_utils import run_bass_kernel_spmd

    cfg = dict(BEST_CFG)
    if any(p != meta["PCs"][0] for p in meta["PCs"]):
        cfg["obatch"] = 1  # obatch packing needs uniform slot capacities
    nc = _build_nc(meta["PCs"], **cfg)
    _LAST.update(nc=nc, in_maps=in_maps, meta=meta)
    res = run_bass_kernel_spmd(nc, in_maps, core_ids=list(range(NCORES)))
    return _gather(res.results, meta)

